# revision 1
# baseline (speedup 1.0000x reference)
"""Trainium2 Bass kernel for nn_BatchDelayProcessor.

Computes, per batch row (B=64, T=441000, D=22050 delay, 20 blocks):
    delayed[t] = 0                          , t < D
    delayed[t] = x[t-D] + 0.3*delayed[t-D]  , t >= D
    out[t]     = 0.5*x[t] + 0.5*delayed[t]

Block recurrence (blocks of D samples):  d_{k+1} = x_k + 0.3*d_k, d_0 = 0;
out_k = 0.5*x_k + 0.5*d_k.  With a scaled carry c_k = 0.5*d_k, two fused
scalar_tensor_tensor ops on the vector engine per block:
    out_k   = (x_k * 0.5)  + c_k
    c_{k+1} = (c_k * -0.7) + out_k         [1 - 0.7 == 0.3 exactly in f32]

Sharding: data-parallel over batch — 8 rows per NeuronCore, 8 cores, no
communication.  Per-core layout: each block (8 rows x 22050) is viewed as
(120 partitions x 1470 f32): partition (r*15+s) holds row r, sub-slice s
(1470 contiguous samples = 5880 B per DMA descriptor row).

Raw Bass (not Tile): Tile's semaphore assignment put 3 sync waits on one
compute instruction, which overflows the walrus codegen per-instruction
sync-wait encoding ("Too many sync wait commands").  Here each wait is a
standalone sequencer wait_ge, and the pipeline is hand-scheduled.

Engine split (HWDGE descriptor generation is the load-side bottleneck at
~5us per 120-descriptor load, so loads use BOTH HWDGE rings):
  SP sequencer:  DMA-in of even blocks   (HWDGE ring 0)
  ACT sequencer: DMA-in of odd blocks    (HWDGE ring 1, no compute)
  GpSimd:        DMA-out of all blocks   (SWDGE)
  DVE:           the two STT ops per block
with ring buffers (x: NX tiles, out: NO tiles, carry: 2 tiles).
"""

from contextlib import ExitStack

import numpy as np

import concourse.bass as bass
import concourse.mybir as mybir
from concourse.bass_utils import run_bass_kernel_spmd

B, T = 64, 441000
D, NBLK = 22050, 20
NCORES = 8
ROWS = B // NCORES          # 8 rows per core
SPLITS = 15                 # 22050 = 15 * 1470
FREE = D // SPLITS          # 1470
P = ROWS * SPLITS           # 120 partitions

NX = 12                     # x-tile ring (DMA-in lookahead > 8 load queues)
NO = 12                     # out-tile ring
NC = 2                      # carry ping-pong

F32 = mybir.dt.float32


def build_nc() -> bass.Bass:
    nc = bass.Bass(trn_type="TRN2")
    x = nc.declare_dram_parameter("x", [ROWS, T], F32, isOutput=False)
    y = nc.declare_dram_parameter("y", [ROWS, T], F32, isOutput=True)
    # (block, row, split, free)
    xv = x.rearrange("r (k s f) -> k r s f", k=NBLK, s=SPLITS)
    yv = y.rearrange("r (k s f) -> k r s f", k=NBLK, s=SPLITS)

    with ExitStack() as ctx:
        block = ctx.enter_context(nc.Block())
        xbuf = ctx.enter_context(nc.sbuf_tensor("xbuf", [P, NX * FREE], F32))
        obuf = ctx.enter_context(nc.sbuf_tensor("obuf", [P, NO * FREE], F32))
        cbuf = ctx.enter_context(nc.sbuf_tensor("cbuf", [P, NC * FREE], F32))
        # Per-ring-slot DMA sems: a slot's next DMA is issued only after the
        # sequencer re-observed the slot sem at its current value, so the
        # async SDMA increments on one sem are never concurrent (same
        # protocol as Tile's DMAHW lanes).
        s_in = [
            ctx.enter_context(nc.semaphore(f"s_in{j}")) for j in range(NX)
        ]
        s_out = [
            ctx.enter_context(nc.semaphore(f"s_out{j}")) for j in range(NO)
        ]
        s_dve = ctx.enter_context(nc.semaphore("s_dve"))

        def slot(buf, k, n):
            j = k % n
            return buf[:, j * FREE : (j + 1) * FREE]

        # DMA pairs the (8,15,1470) DRAM view with the (120,1470) SBUF slot:
        # traversal orders match since partition p = r*15 + s.
        slot3d = slot

        # Completion-count conventions:
        #   s_in[j] : 16*(k//NX + 1) after DMA-in of block k (j = k%NX)
        #   s_out[j]: 16*(k//NO + 1) after DMA-out of block k (j = k%NO)
        #   s_dve   : 1 after memset, 2k+2 after out_k, 2k+3 after c_{k+1}
        # NX/NO are even, so one slot's successive DMAs stay on one
        # sequencer and its slot-sem updates stay ordered.

        def emit_in(eng, k):
            if k >= NX:
                # WAR: xbuf slot k%NX last read by out_{k-NX}'s STT
                eng.wait_ge(s_dve, 2 * (k - NX) + 2)
                # slot sem at its current value (race-free async incs)
                eng.wait_ge(s_in[k % NX], 16 * (k // NX))
            eng.dma_start(out=slot3d(xbuf, k, NX), in_=xv[k]).then_inc(
                s_in[k % NX], 16
            )

        out_counts = [0] * NO

        def emit_out(eng, k):
            j = k % NO
            eng.wait_ge(s_dve, 2 * k + 2)  # out_k ready
            if out_counts[j]:
                eng.wait_ge(s_out[j], 16 * out_counts[j])
            eng.dma_start(out=yv[k], in_=slot3d(obuf, k, NO)).then_inc(
                s_out[j], 16
            )
            out_counts[j] += 1

        @block.sync
        def _(sync):
            for k in range(0, NBLK, 2):
                emit_in(sync, k)

        @block.scalar
        def _(scalar):
            for k in range(1, NBLK, 2):
                emit_in(scalar, k)

        # Stores on the GpSimd SWDGE ring (A/B-tested against putting them
        # on the ACT HWDGE ring alongside the odd loads, and against
        # splitting the final store per-row across queues: both were
        # measurably slower — per-DMA SWDGE emission overhead outweighs
        # the tail it would hide).
        @block.gpsimd
        def _(gpsimd):
            for k in range(NBLK):
                emit_out(gpsimd, k)

        @block.vector
        def _(vector):
            nc.vector.memset(slot(cbuf, 0, NC), 0.0).then_inc(s_dve, 1)
            for k in range(NBLK):
                vector.wait_ge(s_in[k % NX], 16 * (k // NX + 1))  # x_k loaded
                # DVE writes drain async: same-engine RAW on c_k needs a wait
                vector.wait_ge(s_dve, 2 * k + 1)  # c_k drained (memset @ k=0)
                if k >= NO:
                    # WAR: obuf slot k%NO last read by DMA-out of k-NO
                    vector.wait_ge(s_out[k % NO], 16 * (k // NO))
                # out_k = (x_k * 0.5) + c_k
                nc.vector.scalar_tensor_tensor(
                    out=slot(obuf, k, NO),
                    in0=slot(xbuf, k, NX),
                    scalar=0.5,
                    in1=slot(cbuf, k, NC),
                    op0=mybir.AluOpType.mult,
                    op1=mybir.AluOpType.add,
                ).then_inc(s_dve, 1)
                if k < NBLK - 1:
                    vector.wait_ge(s_dve, 2 * k + 2)  # out_k drained
                    # c_{k+1} = (c_k * -0.7) + out_k
                    nc.vector.scalar_tensor_tensor(
                        out=slot(cbuf, k + 1, NC),
                        in0=slot(cbuf, k, NC),
                        scalar=-0.7,
                        in1=slot(obuf, k, NO),
                        op0=mybir.AluOpType.mult,
                        op1=mybir.AluOpType.add,
                    ).then_inc(s_dve, 1)

    return nc


_NC_CACHE = None


def _get_nc() -> bass.Bass:
    global _NC_CACHE
    if _NC_CACHE is None:
        _NC_CACHE = build_nc()
    return _NC_CACHE


def _shard(x: np.ndarray) -> list[dict[str, np.ndarray]]:
    x = np.ascontiguousarray(np.asarray(x, dtype=np.float32))
    assert x.shape == (B, T), x.shape
    return [
        {"x": np.ascontiguousarray(x[i * ROWS : (i + 1) * ROWS])}
        for i in range(NCORES)
    ]


def kernel(x: np.ndarray) -> np.ndarray:
    nc = _get_nc()
    res = run_bass_kernel_spmd(nc, _shard(x), core_ids=list(range(NCORES)))
    return np.concatenate([r["y"] for r in res.results], axis=0)


def kernel_profiled(x: np.ndarray):
    """Like kernel() but with NTFF tracing; returns (out, BassKernelResults)."""
    nc = _get_nc()
    res = run_bass_kernel_spmd(
        nc, _shard(x), core_ids=list(range(NCORES)), trace=True
    )
    out = np.concatenate([r["y"] for r in res.results], axis=0)
    return out, res



# revision 2
# speedup vs baseline: 1.0709x; 1.0709x over previous
"""Trainium2 Bass kernel for nn_BatchDelayProcessor.

Computes, per batch row (B=64, T=441000, D=22050 delay, 20 blocks):
    delayed[t] = 0                          , t < D
    delayed[t] = x[t-D] + 0.3*delayed[t-D]  , t >= D
    out[t]     = 0.5*x[t] + 0.5*delayed[t]

Block recurrence (blocks of D samples):  d_{k+1} = x_k + 0.3*d_k, d_0 = 0;
out_k = 0.5*x_k + 0.5*d_k.  With u_k = 0.5*x_k and the scaled carry
v_k = 0.5*d_k:
    u_k     = 0.5 * x_k                    [ACT copy-activation, f32 -> bf16]
    v_{k+1} = (v_k * 0.3) + u_k            [DVE STT, bf16 -- the ONLY serial op]
    out_k   = u_k + v_k                    [DVE TT,  bf16 -- off the chain]
Only the v-chain (20 ops) is serial; everything else pipelines.  bf16 gets
the DVE 2x perf mode (f32 tensor_tensor is stuck at 1x), and the rel-err
budget (2e-2) dwarfs bf16 rounding (~2e-3 here).

Sharding: data-parallel over batch -- 8 rows per NeuronCore, 8 cores, no
communication.  Per-core layout: each block (8 rows x 22050) is viewed as
(120 partitions x 1470): partition (r*15+s) holds row r, sub-slice s
(1470 contiguous samples).

Raw Bass (not Tile; Tile's semaphore assignment overflowed the per-
instruction sync-wait encoding for this pipeline).  Engine split:
  SP  sequencer: DMA-in of even blocks  (HWDGE ring 0, f32)
  ACT sequencer: DMA-in of odd blocks   (HWDGE ring 1) + u_k casts
  DVE:           v-chain STT + out TT (bf16)
  GpSimd:        DMA-out of all blocks (SWDGE, casts bf16 -> f32 in-DMA)
"""

from contextlib import ExitStack

import numpy as np

import concourse.bass as bass
import concourse.mybir as mybir
from concourse.bass_utils import run_bass_kernel_spmd

B, T = 64, 441000
D, NBLK = 22050, 20
NCORES = 8
ROWS = B // NCORES          # 8 rows per core
SPLITS = 15                 # 22050 = 15 * 1470
FREE = D // SPLITS          # 1470
P = ROWS * SPLITS           # 120 partitions

NX = 12                     # f32 x-tile ring (DMA-in lookahead)
NU = 8                      # bf16 u-tile ring
NO = 12                     # bf16 out-tile ring
NV = 2                      # carry ping-pong
LOOKAHEAD = 4               # odd-load issue lookahead on the ACT sequencer

F32 = mybir.dt.float32
BF16 = mybir.dt.bfloat16


def build_nc() -> bass.Bass:
    nc = bass.Bass(trn_type="TRN2")
    x = nc.declare_dram_parameter("x", [ROWS, T], F32, isOutput=False)
    y = nc.declare_dram_parameter("y", [ROWS, T], F32, isOutput=True)
    # (block, row, split, free)
    xv = x.rearrange("r (k s f) -> k r s f", k=NBLK, s=SPLITS)
    yv = y.rearrange("r (k s f) -> k r s f", k=NBLK, s=SPLITS)

    with ExitStack() as ctx:
        block = ctx.enter_context(nc.Block())
        xbuf = ctx.enter_context(nc.sbuf_tensor("xbuf", [P, NX * FREE], F32))
        ubuf = ctx.enter_context(nc.sbuf_tensor("ubuf", [P, NU * FREE], BF16))
        obuf = ctx.enter_context(nc.sbuf_tensor("obuf", [P, NO * FREE], BF16))
        vbuf = ctx.enter_context(nc.sbuf_tensor("vbuf", [P, NV * FREE], BF16))
        # Per-ring-slot DMA sems: a slot's next DMA is issued only after the
        # sequencer re-observed the slot sem at its current value, so the
        # async SDMA increments on one sem are never concurrent.
        s_in = [
            ctx.enter_context(nc.semaphore(f"s_in{j}")) for j in range(NX)
        ]
        s_out = [
            ctx.enter_context(nc.semaphore(f"s_out{j}")) for j in range(NO)
        ]
        s_u = ctx.enter_context(nc.semaphore("s_u"))
        s_dve = ctx.enter_context(nc.semaphore("s_dve"))

        def slot(buf, k, n):
            j = k % n
            return buf[:, j * FREE : (j + 1) * FREE]

        # Completion-count conventions:
        #   s_in[j] : 16*(k//NX + 1) after DMA-in of block k (j = k%NX)
        #   s_out[j]: 16*(store_count) after DMA-out on slot j
        #   s_u     : k+1 after u_k's cast
        #   s_dve   : see v_done/out_done below
        # v_done[k]   = s_dve value once v_k is written (memset or chain STT)
        # out_done[k] = s_dve value once out_k is written (TT)
        v_done = [0] * (NBLK + 1)
        out_done = [0] * NBLK
        v_done[0] = 1
        cnt = 1
        for k in range(NBLK):
            if k < NBLK - 1:
                cnt += 1
                v_done[k + 1] = cnt
            cnt += 1
            out_done[k] = cnt

        def emit_in(eng, k):
            if k >= NX:
                # WAR: xbuf slot k%NX last read by u_{k-NX}'s cast on ACT
                eng.wait_ge(s_u, k - NX + 1)
                # slot sem at its current value (race-free async incs)
                eng.wait_ge(s_in[k % NX], 16 * (k // NX))
            eng.dma_start(out=slot(xbuf, k, NX), in_=xv[k]).then_inc(
                s_in[k % NX], 16
            )

        @block.sync
        def _(sync):
            for k in range(0, NBLK, 2):
                emit_in(sync, k)

        @block.scalar
        def _(scalar):
            # prologue: first few odd loads so the ring has lookahead
            for k in range(1, min(1 + LOOKAHEAD, NBLK), 2):
                emit_in(scalar, k)
            for k in range(NBLK):
                ka = k + LOOKAHEAD
                if ka % 2 == 1 and ka < NBLK:
                    emit_in(scalar, ka)
                # u_k = bf16(0.5 * x_k)
                scalar.wait_ge(s_in[k % NX], 16 * (k // NX + 1))
                if k >= NU:
                    # WAR: ubuf slot k%NU last read by out_{k-NU}'s TT
                    scalar.wait_ge(s_dve, out_done[k - NU])
                nc.scalar.activation(
                    out=slot(ubuf, k, NU),
                    in_=slot(xbuf, k, NX),
                    func=mybir.ActivationFunctionType.Copy,
                    scale=0.5,
                ).then_inc(s_u, 1)

        # Stores on the GpSimd SWDGE ring (also does the bf16 -> f32 cast;
        # HWDGE rejects dtype-casting DMAs).
        out_counts = [0] * NO

        @block.gpsimd
        def _(gpsimd):
            for k in range(NBLK):
                j = k % NO
                gpsimd.wait_ge(s_dve, out_done[k])  # out_k ready
                if out_counts[j]:
                    gpsimd.wait_ge(s_out[j], 16 * out_counts[j])
                gpsimd.dma_start(out=yv[k], in_=slot(obuf, k, NO)).then_inc(
                    s_out[j], 16
                )
                out_counts[j] += 1

        @block.vector
        def _(vector):
            nc.vector.memset(slot(vbuf, 0, NV), 0.0).then_inc(s_dve, 1)
            for k in range(NBLK):
                vector.wait_ge(s_u, k + 1)  # u_k ready
                if k < NBLK - 1:
                    # v_{k+1} = (v_k * 0.3) + u_k  -- the serial chain.
                    # v_k was written by this engine (program order + drain);
                    # no same-engine wait needed.
                    nc.vector.scalar_tensor_tensor(
                        out=slot(vbuf, k + 1, NV),
                        in0=slot(vbuf, k, NV),
                        scalar=0.3,
                        in1=slot(ubuf, k, NU),
                        op0=mybir.AluOpType.mult,
                        op1=mybir.AluOpType.add,
                    ).then_inc(s_dve, 1)
                if k >= NO:
                    # WAR: obuf slot k%NO last read by DMA-out of k-NO
                    vector.wait_ge(s_out[k % NO], 16 * (k // NO))
                # out_k = u_k + v_k
                nc.vector.tensor_tensor(
                    out=slot(obuf, k, NO),
                    in0=slot(ubuf, k, NU),
                    in1=slot(vbuf, k, NV),
                    op=mybir.AluOpType.add,
                ).then_inc(s_dve, 1)

    return nc


_NC_CACHE = None


def _get_nc() -> bass.Bass:
    global _NC_CACHE
    if _NC_CACHE is None:
        _NC_CACHE = build_nc()
    return _NC_CACHE


def _shard(x: np.ndarray) -> list[dict[str, np.ndarray]]:
    x = np.ascontiguousarray(np.asarray(x, dtype=np.float32))
    assert x.shape == (B, T), x.shape
    return [
        {"x": np.ascontiguousarray(x[i * ROWS : (i + 1) * ROWS])}
        for i in range(NCORES)
    ]


def kernel(x: np.ndarray) -> np.ndarray:
    nc = _get_nc()
    res = run_bass_kernel_spmd(nc, _shard(x), core_ids=list(range(NCORES)))
    return np.concatenate([r["y"] for r in res.results], axis=0)


def kernel_profiled(x: np.ndarray):
    """Like kernel() but with NTFF tracing; returns (out, BassKernelResults)."""
    nc = _get_nc()
    res = run_bass_kernel_spmd(
        nc, _shard(x), core_ids=list(range(NCORES)), trace=True
    )
    out = np.concatenate([r["y"] for r in res.results], axis=0)
    return out, res


# revision 9
# speedup vs baseline: 1.4486x; 1.3526x over previous
"""Trainium2 Bass kernel for nn_BatchDelayProcessor.

Computes, per batch row (B=64, T=441000, D=22050 delay, 20 blocks):
    delayed[t] = 0                          , t < D
    delayed[t] = x[t-D] + 0.3*delayed[t-D]  , t >= D
    out[t]     = 0.5*x[t] + 0.5*delayed[t]

Unrolling the block recurrence, out_p = sum_j W[p,j] * x_j with the banded
lower-triangular W[p,p] = 0.5, W[p,j] = 0.5*0.3^(p-1-j) (j<p) -- i.e. a
20x20 matmul over the block axis, identical for every row.  So: lay out
SBUF as partition = (row, block), free = sample offset, and let the PE do
the whole recurrence as OUT = W @ X with a block-diagonal stationary
(4 rows/group -> 80x80).  float32r runs at full PE rate for moving dims
>= 256, so no input cast is needed.

Why this layout wins: DMA descriptors become one long contiguous DRAM run
per (row, block, column-slab) -- 17.6 KB instead of the 5.9 KB of the
partition=(row, 1/15th-block) STT formulation whose HWDGE descriptor
generation (~64 ns/desc, 2400 descs) capped the kernel at ~118 us.  All
DMA goes through SWDGE (GpSimd): its emission is ~0.8 us per dma_start
regardless of descriptor count, and queue 0 spreads over all 16 SDMA
engines (~355 GB/s) where the two HWDGE rings share only 8 (~200 GB/s).

Per core: 2 row-groups x 5 column-slabs of 4410 samples; 10 matmuls of
441 columns per slab (PSUM bank per matmul, 8 banks round-robin).  PSUM
f32 -> SBUF bf16 copies split DVE (even matmul idx) / ACT (odd idx) --
each is ~1 elem/lane/cycle (PSUM read port), one engine alone would be
the bottleneck.  y is written as bf16 (halves store HBM traffic; rel-err
budget 2e-2 vs bf16's ~2e-3) and upcast to f32 on the host.

Engine split:
  GpSimd: ALL DMA via SWDGE queue 0 (W, slab loads f32, slab stores bf16)
  PE:     100 f32r matmuls (80-partition block-diag W)
  DVE:    PSUM->SBUF bf16 copies, even matmul indices
  ACT:    PSUM->SBUF bf16 copies, odd matmul indices
  SP:     idle
"""

from contextlib import ExitStack

import numpy as np

import concourse.bass as bass
import concourse.mybir as mybir
from concourse.bass_utils import run_bass_kernel_spmd

B, T = 64, 441000
D, NBLK = 22050, 20
NCORES = 8
ROWS = B // NCORES          # 8 rows per core
GROUPS = 2                  # row groups per core
GR = ROWS // GROUPS         # 4 rows per group
P = GR * NBLK               # 80 partitions: (row-in-group, block)
SLAB = 4410                 # columns per slab (per block)
NSLAB = D // SLAB           # 5 slabs per group
MMCOL = 441                 # columns per matmul
MM_PER_SLAB = SLAB // MMCOL  # 10
NBANK = 8                   # PSUM banks in round-robin
NXS = 3                     # xbuf slab ring (per group)
NOS = 2                     # obuf slab ring (per group)

F32 = mybir.dt.float32
F32R = mybir.dt.float32r
BF16 = mybir.dt.bfloat16

# Global slab schedule: alternate groups for an even pipeline.
SLAB_ORDER = [(t % 2, t // 2) for t in range(GROUPS * NSLAB)]
NT = len(SLAB_ORDER)        # 10
NMM = NT * MM_PER_SLAB      # 100


def _weights() -> np.ndarray:
    """lhsT for nc.tensor.matmul: out = lhsT.T @ rhs.

    lhsT[(r,j), (r',p)] = W[p, j] if r == r' else 0, with
    W[p, j] = 0.5*(p==j) + 0.5*0.3^(p-1-j)*(j<p).
    """
    W = np.zeros((NBLK, NBLK), np.float64)
    for p in range(NBLK):
        W[p, p] = 0.5
        for j in range(p):
            W[p, j] = 0.5 * 0.3 ** (p - 1 - j)
    return np.kron(np.eye(GR), W.T).astype(np.float32)


def build_nc() -> bass.Bass:
    nc = bass.Bass(trn_type="TRN2")
    x = nc.declare_dram_parameter("x", [ROWS, T], F32, isOutput=False)
    w = nc.declare_dram_parameter("w", [P, P], F32, isOutput=False)
    y = nc.declare_dram_parameter("y", [ROWS, T], BF16, isOutput=True)
    xv = x.rearrange("r (j c) -> r j c", j=NBLK)   # (8, 20, 22050)
    yv = y.rearrange("r (j c) -> r j c", j=NBLK)

    with ExitStack() as ctx:
        block = ctx.enter_context(nc.Block())
        wbuf = ctx.enter_context(nc.sbuf_tensor("wbuf", [P, P], BF16))
        xbuf = [
            ctx.enter_context(
                nc.sbuf_tensor(f"xbuf{g}", [P, NXS * SLAB], BF16)
            )
            for g in range(GROUPS)
        ]
        obuf = [
            ctx.enter_context(nc.sbuf_tensor(f"obuf{g}", [P, NOS * SLAB], BF16))
            for g in range(GROUPS)
        ]
        psum = [
            ctx.enter_context(nc.psum_tensor(f"ps{b}", [P, MMCOL], F32))
            for b in range(NBANK)
        ]
        s_w = ctx.enter_context(nc.semaphore("s_w"))
        s_x = [
            [ctx.enter_context(nc.semaphore(f"s_x{g}_{j}")) for j in range(NXS)]
            for g in range(GROUPS)
        ]
        s_o = [
            [ctx.enter_context(nc.semaphore(f"s_o{g}_{j}")) for j in range(NOS)]
            for g in range(GROUPS)
        ]
        s_mm = ctx.enter_context(nc.semaphore("s_mm"))
        s_cpd = ctx.enter_context(nc.semaphore("s_cpd"))
        s_cpa = ctx.enter_context(nc.semaphore("s_cpa"))

        def xslab(g, s):
            j = s % NXS
            return xbuf[g][:, j * SLAB : (j + 1) * SLAB]

        def oslab(g, s):
            j = s % NOS
            return obuf[g][:, j * SLAB : (j + 1) * SLAB]

        # copies done counts: copy idx -> (# s_cpd incs, # s_cpa incs) after it
        def copies_done(last_idx):
            return (last_idx + 2) // 2, (last_idx + 1) // 2

        def _store(gp, t):
            g, s = SLAB_ORDER[t]
            nd, na = copies_done((t + 1) * MM_PER_SLAB - 1)
            gp.wait_ge(s_cpd, nd)
            gp.wait_ge(s_cpa, na)
            if s >= NOS:
                gp.wait_ge(s_o[g][s % NOS], 16 * (s // NOS))
            gp.dma_start(
                out=yv[g * GR : (g + 1) * GR, :, s * SLAB : (s + 1) * SLAB],
                in_=oslab(g, s),
            ).then_inc(s_o[g][s % NOS], 16)

        @block.gpsimd
        def _(gp):
            gp.dma_start(out=wbuf[:, :], in_=w[:, :]).then_inc(
                s_w, 16
            )
            for t, (g, s) in enumerate(SLAB_ORDER):
                if s >= NXS:
                    # xbuf slot WAR: all matmuls of slab (g, s-NXS) retired
                    t_old = 2 * (s - NXS) + g
                    gp.wait_ge(s_mm, (t_old + 1) * MM_PER_SLAB)
                    gp.wait_ge(s_x[g][s % NXS], 16 * (s // NXS))
                gp.dma_start(
                    out=xslab(g, s),
                    in_=xv[
                        g * GR : (g + 1) * GR, :, s * SLAB : (s + 1) * SLAB
                    ],
                ).then_inc(s_x[g][s % NXS], 16)
                if t >= 2:
                    _store(gp, t - 2)
            _store(gp, NT - 2)
            _store(gp, NT - 1)

        @block.tensor
        def _(te):
            te.wait_ge(s_w, 16)
            for t, (g, s) in enumerate(SLAB_ORDER):
                for i in range(MM_PER_SLAB):
                    idx = t * MM_PER_SLAB + i
                    if i == 0:
                        te.wait_ge(s_x[g][s % NXS], 16 * (s // NXS + 1))
                    if idx >= NBANK:
                        # PSUM bank WAR: copy idx-NBANK retired.  Bank b is
                        # always drained by the same engine (NBANK even).
                        old = idx - NBANK
                        if old % 2 == 0:
                            te.wait_ge(s_cpd, old // 2 + 1)
                        else:
                            te.wait_ge(s_cpa, old // 2 + 1)
                    c0 = (s % NXS) * SLAB + i * MMCOL
                    nc.tensor.matmul(
                        out=psum[idx % NBANK][:, :],
                        lhsT=wbuf[:, :],
                        rhs=xbuf[g][:, c0 : c0 + MMCOL],
                        start=True,
                        stop=True,
                    ).then_inc(s_mm, 1)

        def _copy_prog(eng, vec, parity, sem):
            for t, (g, s) in enumerate(SLAB_ORDER):
                for i in range(MM_PER_SLAB):
                    idx = t * MM_PER_SLAB + i
                    if idx % 2 != parity:
                        continue
                    eng.wait_ge(s_mm, idx + 1)
                    if i == parity and s >= NOS:
                        # obuf slot WAR on this engine's first chunk of the
                        # slab: store of slab (g, s-NOS) retired
                        eng.wait_ge(s_o[g][s % NOS], 16 * (s // NOS))
                    dst = obuf[g][
                        :,
                        (s % NOS) * SLAB + i * MMCOL : (s % NOS) * SLAB
                        + (i + 1) * MMCOL,
                    ]
                    vec(dst, psum[idx % NBANK][:, :]).then_inc(sem, 1)

        @block.vector
        def _(ve):
            _copy_prog(ve, nc.vector.tensor_copy, 0, s_cpd)

        @block.scalar
        def _(sc):
            _copy_prog(sc, nc.scalar.copy, 1, s_cpa)

    return nc


_NC_CACHE = None


def _get_nc() -> bass.Bass:
    global _NC_CACHE
    if _NC_CACHE is None:
        _NC_CACHE = build_nc()
    return _NC_CACHE


_W = _weights()


def _shard(x: np.ndarray) -> list[dict[str, np.ndarray]]:
    x = np.ascontiguousarray(np.asarray(x, dtype=np.float32))
    assert x.shape == (B, T), x.shape
    return [
        {
            "x": np.ascontiguousarray(x[i * ROWS : (i + 1) * ROWS]),
            "w": _W,
        }
        for i in range(NCORES)
    ]


def kernel(x: np.ndarray) -> np.ndarray:
    nc = _get_nc()
    res = run_bass_kernel_spmd(nc, _shard(x), core_ids=list(range(NCORES)))
    return np.concatenate(
        [np.asarray(r["y"]) for r in res.results], axis=0
    ).astype(np.float32)


def kernel_profiled(x: np.ndarray):
    """Like kernel() but with NTFF tracing; returns (out, BassKernelResults)."""
    nc = _get_nc()
    res = run_bass_kernel_spmd(
        nc, _shard(x), core_ids=list(range(NCORES)), trace=True
    )
    out = np.concatenate(
        [np.asarray(r["y"]) for r in res.results], axis=0
    ).astype(np.float32)
    return out, res


# revision 11
# speedup vs baseline: 1.6048x; 1.1078x over previous
"""Trainium2 Bass kernel for nn_BatchDelayProcessor.

Computes, per batch row (B=64, T=441000, D=22050 delay, 20 blocks):
    delayed[t] = 0                          , t < D
    delayed[t] = x[t-D] + 0.3*delayed[t-D]  , t >= D
    out[t]     = 0.5*x[t] + 0.5*delayed[t]

Unrolling the block recurrence, out_p = sum_j W[p,j] * x_j with the banded
lower-triangular W[p,p] = 0.5, W[p,j] = 0.5*0.3^(p-1-j) (j<p) -- i.e. a
20x20 matmul over the block axis, identical for every row.  So: lay out
SBUF as partition = (row, block), free = sample offset, and let the PE do
the whole recurrence as OUT = W @ X with a block-diagonal stationary
(4 rows/group -> 80x80), bf16 in / f32 PSUM out.

Why this layout wins: DMA descriptors become one long contiguous DRAM run
per (row, block, column-slab) -- 17.6 KB reads instead of the 5.9 KB of
the partition=(row, 1/15th-block) STT formulation whose HWDGE descriptor
generation (~64 ns/desc, 2400 descs) capped the kernel at ~118 us.  All
DMA goes through SWDGE (GpSimd): its emission is ~0.8 us per dma_start
regardless of descriptor count, and queue 0 spreads over all 16 SDMA
engines (~355 GB/s) where the two HWDGE rings share only 8 (~200 GB/s).
The f32->bf16 input cast happens inside the load DMA (SWDGE-only
feature), so no engine pass is spent on it.

Per core: 2 row-groups x 5 column-slabs of 4410 samples, fully buffered
in SBUF (x: 88 KB/partition bf16, out: 88 KB) so loads are emitted
back-to-back with NO waits and stores never gate compute.  9 matmuls of
490 columns per slab (PSUM bank per matmul, 8 banks round-robin).  PSUM
f32 -> SBUF bf16 copies split DVE (even matmul idx) / ACT (odd idx) --
PSUM reads are 1 elem/lane/cycle, one engine alone would bottleneck.
y is written as bf16 (halves store HBM traffic; rel-err budget 2e-2 vs
bf16's ~2e-3) and upcast to f32 on the host.

Engine split:
  GpSimd: ALL DMA via SWDGE queue 0 (W, 10 slab loads, then 10 stores)
  PE:     90 bf16 matmuls (80-partition block-diag W)
  DVE:    PSUM->SBUF bf16 copies, even matmul indices
  ACT:    PSUM->SBUF bf16 copies, odd matmul indices
  SP:     idle
"""

from contextlib import ExitStack

import numpy as np

import concourse.bass as bass
import concourse.mybir as mybir
from concourse.bass_utils import run_bass_kernel_spmd

B, T = 64, 441000
D, NBLK = 22050, 20
NCORES = 8
ROWS = B // NCORES          # 8 rows per core
GROUPS = 2                  # row groups per core
GR = ROWS // GROUPS         # 4 rows per group
P = GR * NBLK               # 80 partitions: (row-in-group, block)
SLAB = 4410                 # columns per slab (per block)
NSLAB = D // SLAB           # 5 slabs per group
MMCOL = 490                 # columns per matmul (<=512 psum bank cap)
MM_PER_SLAB = SLAB // MMCOL  # 9
NBANK = 8                   # PSUM banks in round-robin

F32 = mybir.dt.float32
BF16 = mybir.dt.bfloat16

# Global slab schedule: alternate groups for an even pipeline.
SLAB_ORDER = [(t % 2, t // 2) for t in range(GROUPS * NSLAB)]
NT = len(SLAB_ORDER)        # 10
NMM = NT * MM_PER_SLAB      # 90


def _weights() -> np.ndarray:
    """lhsT for nc.tensor.matmul: out = lhsT.T @ rhs.

    lhsT[(r,j), (r',p)] = W[p, j] if r == r' else 0, with
    W[p, j] = 0.5*(p==j) + 0.5*0.3^(p-1-j)*(j<p).
    """
    W = np.zeros((NBLK, NBLK), np.float64)
    for p in range(NBLK):
        W[p, p] = 0.5
        for j in range(p):
            W[p, j] = 0.5 * 0.3 ** (p - 1 - j)
    return np.kron(np.eye(GR), W.T).astype(np.float32)


def build_nc() -> bass.Bass:
    nc = bass.Bass(trn_type="TRN2")
    x = nc.declare_dram_parameter("x", [ROWS, T], F32, isOutput=False)
    w = nc.declare_dram_parameter("w", [P, P], F32, isOutput=False)
    y = nc.declare_dram_parameter("y", [ROWS, T], BF16, isOutput=True)
    xv = x.rearrange("r (j c) -> r j c", j=NBLK)   # (8, 20, 22050)
    yv = y.rearrange("r (j c) -> r j c", j=NBLK)

    with ExitStack() as ctx:
        block = ctx.enter_context(nc.Block())
        wbuf = ctx.enter_context(nc.sbuf_tensor("wbuf", [P, P], BF16))
        # Full group resident: slab s of group g lives at columns [s*SLAB, ...)
        xbuf = [
            ctx.enter_context(
                nc.sbuf_tensor(f"xbuf{g}", [P, NSLAB * SLAB], BF16)
            )
            for g in range(GROUPS)
        ]
        obuf = [
            ctx.enter_context(
                nc.sbuf_tensor(f"obuf{g}", [P, NSLAB * SLAB], BF16)
            )
            for g in range(GROUPS)
        ]
        psum = [
            ctx.enter_context(nc.psum_tensor(f"ps{b}", [P, MMCOL], F32))
            for b in range(NBANK)
        ]
        s_w = ctx.enter_context(nc.semaphore("s_w"))
        s_x = [
            [
                ctx.enter_context(nc.semaphore(f"s_x{g}_{s}"))
                for s in range(NSLAB)
            ]
            for g in range(GROUPS)
        ]
        s_mm = ctx.enter_context(nc.semaphore("s_mm"))
        s_cpd = ctx.enter_context(nc.semaphore("s_cpd"))
        s_cpa = ctx.enter_context(nc.semaphore("s_cpa"))

        def xslab(g, s):
            return xbuf[g][:, s * SLAB : (s + 1) * SLAB]

        def oslab(g, s):
            return obuf[g][:, s * SLAB : (s + 1) * SLAB]

        # copies done counts: copy idx -> (# s_cpd incs, # s_cpa incs) after it
        def copies_done(last_idx):
            return (last_idx + 2) // 2, (last_idx + 1) // 2

        @block.gpsimd
        def _(gp):
            gp.dma_start(out=wbuf[:, :], in_=w[:, :]).then_inc(s_w, 16)
            # All loads up front, zero waits: the whole group is resident.
            for g, s in SLAB_ORDER:
                gp.dma_start(
                    out=xslab(g, s),
                    in_=xv[
                        g * GR : (g + 1) * GR, :, s * SLAB : (s + 1) * SLAB
                    ],
                ).then_inc(s_x[g][s], 16)
            # Stores drain as each slab's copies retire; nothing waits on
            # these (no obuf reuse), the Block-exit drain ensures completion.
            for t, (g, s) in enumerate(SLAB_ORDER):
                nd, na = copies_done((t + 1) * MM_PER_SLAB - 1)
                gp.wait_ge(s_cpd, nd)
                gp.wait_ge(s_cpa, na)
                # then_inc only because DGE requires sync info; nothing
                # waits past 16 (the Block-exit drain covers completion).
                gp.dma_start(
                    out=yv[
                        g * GR : (g + 1) * GR, :, s * SLAB : (s + 1) * SLAB
                    ],
                    in_=oslab(g, s),
                ).then_inc(s_x[g][s], 16)

        @block.tensor
        def _(te):
            te.wait_ge(s_w, 16)
            for t, (g, s) in enumerate(SLAB_ORDER):
                for i in range(MM_PER_SLAB):
                    idx = t * MM_PER_SLAB + i
                    if i == 0:
                        te.wait_ge(s_x[g][s], 16)
                    if idx >= NBANK:
                        # PSUM bank WAR: copy idx-NBANK retired
                        old = idx - NBANK
                        if old % 2 == 0:
                            te.wait_ge(s_cpd, old // 2 + 1)
                        else:
                            te.wait_ge(s_cpa, old // 2 + 1)
                    c0 = s * SLAB + i * MMCOL
                    nc.tensor.matmul(
                        out=psum[idx % NBANK][:, :],
                        lhsT=wbuf[:, :],
                        rhs=xbuf[g][:, c0 : c0 + MMCOL],
                        start=True,
                        stop=True,
                    ).then_inc(s_mm, 1)

        def _copy_prog(eng, vec, parity, sem):
            for t, (g, s) in enumerate(SLAB_ORDER):
                for i in range(MM_PER_SLAB):
                    idx = t * MM_PER_SLAB + i
                    if idx % 2 != parity:
                        continue
                    eng.wait_ge(s_mm, idx + 1)
                    c0 = s * SLAB + i * MMCOL
                    vec(
                        obuf[g][:, c0 : c0 + MMCOL],
                        psum[idx % NBANK][:, :],
                    ).then_inc(sem, 1)

        @block.vector
        def _(ve):
            _copy_prog(ve, nc.vector.tensor_copy, 0, s_cpd)

        @block.scalar
        def _(sc):
            _copy_prog(sc, nc.scalar.copy, 1, s_cpa)

    return nc


_NC_CACHE = None


def _get_nc() -> bass.Bass:
    global _NC_CACHE
    if _NC_CACHE is None:
        _NC_CACHE = build_nc()
    return _NC_CACHE


_W = _weights()


def _shard(x: np.ndarray) -> list[dict[str, np.ndarray]]:
    x = np.ascontiguousarray(np.asarray(x, dtype=np.float32))
    assert x.shape == (B, T), x.shape
    return [
        {
            "x": np.ascontiguousarray(x[i * ROWS : (i + 1) * ROWS]),
            "w": _W,
        }
        for i in range(NCORES)
    ]


def kernel(x: np.ndarray) -> np.ndarray:
    nc = _get_nc()
    res = run_bass_kernel_spmd(nc, _shard(x), core_ids=list(range(NCORES)))
    return np.concatenate(
        [np.asarray(r["y"]) for r in res.results], axis=0
    ).astype(np.float32)


def kernel_profiled(x: np.ndarray):
    """Like kernel() but with NTFF tracing; returns (out, BassKernelResults)."""
    nc = _get_nc()
    res = run_bass_kernel_spmd(
        nc, _shard(x), core_ids=list(range(NCORES)), trace=True
    )
    out = np.concatenate(
        [np.asarray(r["y"]) for r in res.results], axis=0
    ).astype(np.float32)
    return out, res


# revision 12
# speedup vs baseline: 1.6567x; 1.0323x over previous
"""Trainium2 Bass kernel for nn_BatchDelayProcessor.

Computes, per batch row (B=64, T=441000, D=22050 delay, 20 blocks):
    delayed[t] = 0                          , t < D
    delayed[t] = x[t-D] + 0.3*delayed[t-D]  , t >= D
    out[t]     = 0.5*x[t] + 0.5*delayed[t]

Unrolling the block recurrence, out_p = sum_j W[p,j] * x_j with the banded
lower-triangular W[p,p] = 0.5, W[p,j] = 0.5*0.3^(p-1-j) (j<p) -- i.e. a
20x20 matmul over the block axis, identical for every row.  So: lay out
SBUF as partition = (row, block), free = sample offset, and let the PE do
the whole recurrence as OUT = W @ X with a block-diagonal stationary
(4 rows/group -> 80x80), bf16 in / f32 PSUM out.

Why this layout wins: DMA descriptors become one long contiguous DRAM run
per (row, block, column-slab) -- 17.6 KB reads instead of the 5.9 KB of
the partition=(row, 1/15th-block) STT formulation whose HWDGE descriptor
generation (~64 ns/desc, 2400 descs) capped the kernel at ~118 us.  All
DMA goes through SWDGE (GpSimd): its emission is ~0.8 us per dma_start
regardless of descriptor count, and queue 0 spreads over all 16 SDMA
engines (~355 GB/s) where the two HWDGE rings share only 8 (~200 GB/s).
The f32->bf16 input cast happens inside the load DMA (SWDGE-only
feature), so no engine pass is spent on it.

Per core: 2 row-groups x 5 column-slabs of 4410 samples, fully buffered
in SBUF (x: 88 KB/partition bf16, out: 88 KB) so loads are emitted
back-to-back with NO waits and stores never gate compute.  9 matmuls of
490 columns per slab (PSUM bank per matmul, 8 banks round-robin).  PSUM
f32 -> SBUF bf16 copies split DVE (even matmul idx) / ACT (odd idx) --
PSUM reads are 1 elem/lane/cycle, one engine alone would bottleneck.
y is written as bf16 (halves store HBM traffic; rel-err budget 2e-2 vs
bf16's ~2e-3) and upcast to f32 on the host.

Engine split:
  GpSimd: ALL DMA via SWDGE queue 0 (W, 10 slab loads, then 10 stores)
  PE:     90 bf16 matmuls (80-partition block-diag W)
  DVE:    PSUM->SBUF bf16 copies, even matmul indices
  ACT:    PSUM->SBUF bf16 copies, odd matmul indices
  SP:     idle
"""

from contextlib import ExitStack

import numpy as np

import concourse.bass as bass
import concourse.mybir as mybir
from concourse.bass_utils import run_bass_kernel_spmd

B, T = 64, 441000
D, NBLK = 22050, 20
NCORES = 8
ROWS = B // NCORES          # 8 rows per core
GROUPS = 2                  # row groups per core
GR = ROWS // GROUPS         # 4 rows per group
P = GR * NBLK               # 80 partitions: (row-in-group, block)
SLAB = 4410                 # columns per slab (per block)
NSLAB = D // SLAB           # 5 slabs per group
MMCOL = 490                 # columns per matmul (<=512 psum bank cap)
MM_PER_SLAB = SLAB // MMCOL  # 9
NBANK = 8                   # PSUM banks in round-robin

F32 = mybir.dt.float32
BF16 = mybir.dt.bfloat16

# Global slab schedule: alternate groups for an even pipeline.
SLAB_ORDER = [(t % 2, t // 2) for t in range(GROUPS * NSLAB)]
NT = len(SLAB_ORDER)        # 10
NMM = NT * MM_PER_SLAB      # 90


def _weights() -> np.ndarray:
    """lhsT for nc.tensor.matmul: out = lhsT.T @ rhs.

    lhsT[(r,j), (r',p)] = W[p, j] if r == r' else 0, with
    W[p, j] = 0.5*(p==j) + 0.5*0.3^(p-1-j)*(j<p).
    """
    W = np.zeros((NBLK, NBLK), np.float64)
    for p in range(NBLK):
        W[p, p] = 0.5
        for j in range(p):
            W[p, j] = 0.5 * 0.3 ** (p - 1 - j)
    import ml_dtypes

    return np.kron(np.eye(GR), W.T).astype(ml_dtypes.bfloat16)


def build_nc() -> bass.Bass:
    nc = bass.Bass(trn_type="TRN2")
    x = nc.declare_dram_parameter("x", [ROWS, T], F32, isOutput=False)
    w = nc.declare_dram_parameter("w", [P, P], BF16, isOutput=False)
    y = nc.declare_dram_parameter("y", [ROWS, T], BF16, isOutput=True)
    xv = x.rearrange("r (j c) -> r j c", j=NBLK)   # (8, 20, 22050)
    yv = y.rearrange("r (j c) -> r j c", j=NBLK)

    with ExitStack() as ctx:
        block = ctx.enter_context(nc.Block())
        wbuf = ctx.enter_context(nc.sbuf_tensor("wbuf", [P, P], BF16))
        # Full group resident: slab s of group g lives at columns [s*SLAB, ...)
        xbuf = [
            ctx.enter_context(
                nc.sbuf_tensor(f"xbuf{g}", [P, NSLAB * SLAB], BF16)
            )
            for g in range(GROUPS)
        ]
        obuf = [
            ctx.enter_context(
                nc.sbuf_tensor(f"obuf{g}", [P, NSLAB * SLAB], BF16)
            )
            for g in range(GROUPS)
        ]
        psum = [
            ctx.enter_context(nc.psum_tensor(f"ps{b}", [P, MMCOL], F32))
            for b in range(NBANK)
        ]
        s_w = ctx.enter_context(nc.semaphore("s_w"))
        s_x = [
            [
                ctx.enter_context(nc.semaphore(f"s_x{g}_{s}"))
                for s in range(NSLAB)
            ]
            for g in range(GROUPS)
        ]
        s_mm = ctx.enter_context(nc.semaphore("s_mm"))
        s_cpd = ctx.enter_context(nc.semaphore("s_cpd"))
        s_cpa = ctx.enter_context(nc.semaphore("s_cpa"))

        def xslab(g, s):
            return xbuf[g][:, s * SLAB : (s + 1) * SLAB]

        def oslab(g, s):
            return obuf[g][:, s * SLAB : (s + 1) * SLAB]

        # copies done counts: copy idx -> (# s_cpd incs, # s_cpa incs) after it
        def copies_done(last_idx):
            return (last_idx + 2) // 2, (last_idx + 1) // 2

        @block.sync
        def _(sp):
            # W rides the otherwise-idle SP HWDGE ring, overlapping the
            # ~8.5us GpSimd engine preamble that delays all SWDGE traffic.
            sp.dma_start(out=wbuf[:, :], in_=w[:, :]).then_inc(s_w, 16)

        @block.gpsimd
        def _(gp):
            # All loads up front, zero waits: the whole group is resident.
            for g, s in SLAB_ORDER:
                gp.dma_start(
                    out=xslab(g, s),
                    in_=xv[
                        g * GR : (g + 1) * GR, :, s * SLAB : (s + 1) * SLAB
                    ],
                ).then_inc(s_x[g][s], 16)
            # Stores drain as each slab's copies retire; nothing waits on
            # these (no obuf reuse), the Block-exit drain ensures completion.
            for t, (g, s) in enumerate(SLAB_ORDER):
                nd, na = copies_done((t + 1) * MM_PER_SLAB - 1)
                gp.wait_ge(s_cpd, nd)
                gp.wait_ge(s_cpa, na)
                # then_inc only because DGE requires sync info; nothing
                # waits past 16 (the Block-exit drain covers completion).
                gp.dma_start(
                    out=yv[
                        g * GR : (g + 1) * GR, :, s * SLAB : (s + 1) * SLAB
                    ],
                    in_=oslab(g, s),
                ).then_inc(s_x[g][s], 16)

        @block.tensor
        def _(te):
            te.wait_ge(s_w, 16)
            for t, (g, s) in enumerate(SLAB_ORDER):
                for i in range(MM_PER_SLAB):
                    idx = t * MM_PER_SLAB + i
                    if i == 0:
                        te.wait_ge(s_x[g][s], 16)
                    if idx >= NBANK:
                        # PSUM bank WAR: copy idx-NBANK retired
                        old = idx - NBANK
                        if old % 2 == 0:
                            te.wait_ge(s_cpd, old // 2 + 1)
                        else:
                            te.wait_ge(s_cpa, old // 2 + 1)
                    c0 = s * SLAB + i * MMCOL
                    nc.tensor.matmul(
                        out=psum[idx % NBANK][:, :],
                        lhsT=wbuf[:, :],
                        rhs=xbuf[g][:, c0 : c0 + MMCOL],
                        start=True,
                        stop=True,
                    ).then_inc(s_mm, 1)

        def _copy_prog(eng, vec, parity, sem):
            for t, (g, s) in enumerate(SLAB_ORDER):
                for i in range(MM_PER_SLAB):
                    idx = t * MM_PER_SLAB + i
                    if idx % 2 != parity:
                        continue
                    eng.wait_ge(s_mm, idx + 1)
                    c0 = s * SLAB + i * MMCOL
                    vec(
                        obuf[g][:, c0 : c0 + MMCOL],
                        psum[idx % NBANK][:, :],
                    ).then_inc(sem, 1)

        @block.vector
        def _(ve):
            _copy_prog(ve, nc.vector.tensor_copy, 0, s_cpd)

        @block.scalar
        def _(sc):
            _copy_prog(sc, nc.scalar.copy, 1, s_cpa)

    return nc


_NC_CACHE = None


def _get_nc() -> bass.Bass:
    global _NC_CACHE
    if _NC_CACHE is None:
        _NC_CACHE = build_nc()
    return _NC_CACHE


_W = _weights()


def _shard(x: np.ndarray) -> list[dict[str, np.ndarray]]:
    x = np.ascontiguousarray(np.asarray(x, dtype=np.float32))
    assert x.shape == (B, T), x.shape
    return [
        {
            "x": np.ascontiguousarray(x[i * ROWS : (i + 1) * ROWS]),
            "w": _W,
        }
        for i in range(NCORES)
    ]


def kernel(x: np.ndarray) -> np.ndarray:
    nc = _get_nc()
    res = run_bass_kernel_spmd(nc, _shard(x), core_ids=list(range(NCORES)))
    return np.concatenate(
        [np.asarray(r["y"]) for r in res.results], axis=0
    ).astype(np.float32)


def kernel_profiled(x: np.ndarray):
    """Like kernel() but with NTFF tracing; returns (out, BassKernelResults)."""
    nc = _get_nc()
    res = run_bass_kernel_spmd(
        nc, _shard(x), core_ids=list(range(NCORES)), trace=True
    )
    out = np.concatenate(
        [np.asarray(r["y"]) for r in res.results], axis=0
    ).astype(np.float32)
    return out, res
